# revision 34
# baseline (speedup 1.0000x reference)
"""DigitCaps dynamic-routing kernel for 8 Trainium2 NeuronCores.

Strategy (v2): shard the routes dimension R=1024 across the 8 cores (128
routes per core).  u_hat is never materialized: each routing iteration
computes its weighted route-sum

    s[b,c,o] = sum_{r,i} x[b,r,i] * (c_ij[r,c] * W[r,c,o,i])

directly on the PE as 16 accumulating fp16 matmuls.  The per-core partial
s is combined across cores with an AllGather + on-core binary-tree adds
(iters 0,1; ~2x cheaper than AllReduce on one chip) and an AllToAll +
local tree-reduce for the final iteration (each core then squashes only
its own 32-batch output shard).  The agreement update

    agree[r,c] = sum_{b,o,i} W[r,c,o,i] * x[b,r,i] * v[b,c,o]

is computed locally via G[r,i,c,o] = sum_b x[b,r,i]*v[b,c,o] (PE matmuls)
followed by a multiply + group-reduce on the vector engine.

Activation-table discipline: iters 0/1 compute sqrt(q) as exp(0.5*ln q)
so the scalar engine only ever needs the ln/exp table (shared with the
softmax exp); the exact-sqrt table for the final squash is pre-loaded by
a dummy op that hides under the AllToAll window.  x is supplied twice
from the host (b-major and r-major) so no on-device transposes are
needed.
"""

import math
import sys

for _p in ("/opt/trn_rl_repo",):
    if _p not in sys.path:
        sys.path.insert(0, _p)

import numpy as np

import concourse.bass as bass
import concourse.bacc as bacc
import concourse.mybir as mybir
import concourse.tile as tile
from concourse.bass_utils import run_bass_kernel_spmd

F32 = mybir.dt.float32
F16 = mybir.dt.float16

B, R, C, O, I = 256, 1024, 10, 16, 8
NCORES = 8
RS = R // NCORES          # routes per core
CO = C * O                # 160
COI = C * O * I           # 1280
BS = B // NCORES          # output batch shard per core
NITER = 3
A0 = 0.1                  # softmax(0) over C=10 entries
# pacer counts target ~65% of the modeled PE-idle window: undershoot only
# costs the 1.2GHz->2.4GHz delta on the next matmul block, while overshoot
# delays it outright (the dummies queue in-order ahead of it)
PACE_AG = 62              # mm-end .. G-start window (~10us modeled)
PACE_A2A = 20             # AllToAll window (nothing queued behind)
PACE_MID = 30             # agree/softmax stretch (~5us modeled)


def build_nc(reps=1, niter=NITER, fake_cc=False, chain=False, pace=True,
             pace_ag=PACE_AG, pace_mid=PACE_MID, pace_a2a=PACE_A2A):
    nc = bacc.Bacc(
        "TRN2", target_bir_lowering=False, debug=False, num_devices=NCORES
    )
    xt_d = nc.dram_tensor("xt", [RS, 2 * I * 128], F16, kind="ExternalInput")
    xs_d = nc.dram_tensor("xs", [128, 2 * I * RS], F16, kind="ExternalInput")
    ws_d = nc.dram_tensor("ws", [RS, COI], F16, kind="ExternalInput")
    # W in (c, i, o) layout: packed innermost match for the agree multiply
    wst_d = nc.dram_tensor("wst", [RS, COI], F16, kind="ExternalInput")
    # [16, 2*CO]: row r = batches (16k + r, 128 + 16k + r) for core k —
    # the A2A partition-shard layout; kernel() un-permutes on the host.
    out_d = nc.dram_tensor("vout", [16, 2 * CO], F32, kind="ExternalOutput")

    with tile.TileContext(nc) as tc:
        with (
            tc.tile_pool(name="main", bufs=1) as pool,
            tc.tile_pool(name="ps", bufs=1, space=bass.MemorySpace.PSUM) as ps,
            tc.tile_pool(name="pg", bufs=1, space=bass.MemorySpace.PSUM) as pg,
            tc.tile_pool(name="dram", bufs=1, space="DRAM") as dram,
        ):
            pools = (pool, ps, pg, dram)
            for rep in range(reps):
                _build_body(nc, tc, pools, xt_d, xs_d, ws_d, wst_d, out_d, rep,
                            niter=niter, fake_cc=fake_cc,
                            chain=chain and rep > 0, pace=pace,
                            pace_ag=pace_ag, pace_mid=pace_mid,
                            pace_a2a=pace_a2a)
    nc.finalize()
    _unify_act_tables(nc)
    return nc


def _unify_act_tables(nc):
    """Point every activation-table load at the one table covering all the
    functions this kernel uses (ln, exp, copy), then drop redundant loads.

    The builtin insertion pass picks the first table containing each
    function (ln -> natural_log, exp -> exp_and_others), which forces a
    ~1.3us table reload at every ln<->exp transition on the scalar engine.
    All loads it inserts are dependency-free queue-order instructions, so
    rewriting ids and deleting duplicates is safe.
    """
    from concourse.hw_specs import get_activation_tables

    need = {
        mybir.ActivationFunctionType.Ln,
        mybir.ActivationFunctionType.Exp,
        mybir.ActivationFunctionType.Copy,
    }
    try:
        tabs = get_activation_tables(nc.m.arch)
        names = list(tabs)
        target = names.index("natural_log_exp_and_others")
        if not need <= tabs["natural_log_exp_and_others"]:
            return
    except Exception:
        # unknown act_info layout: leave the stock (slower) table loads
        return
    cur = None
    for bb in nc.main_func.blocks:
        keep = []
        for inst in bb.instructions:
            if isinstance(inst, mybir.InstLoadActFuncSet):
                inst.act_func_set_id = target
                if cur == target:
                    continue
                cur = target
            elif isinstance(inst, mybir.InstActivation):
                assert inst.func in need, f"unexpected ACT func {inst.func}"
            keep.append(inst)
        bb.instructions[:] = keep


def _build_body(nc, tc, pools, xt_d, xs_d, ws_d, wst_d, out_d, rep, niter=NITER,
                fake_cc=False, chain=False, pace=True, pace_ag=PACE_AG,
                pace_mid=PACE_MID, pace_a2a=PACE_A2A):
    pool, ps, pg, dram = pools
    rg = [list(range(NCORES))]
    rp = f"r{rep}_"

    def _pace_pe(dummy_ps, xt, n):
        # Keep the PE p-state streak alive across collective/DVE windows:
        # self-paced throwaway matmuls into a scratch PSUM bank.  No
        # consumers; they only read xt, so they fill PE idle time without
        # delaying ready work by more than one dummy's tail (~110ns).
        for _ in range(n):
            nc.tensor.matmul(
                dummy_ps[:], xt[:, 0:128], xt[:, 0:256],
                start=True, stop=True, skip_group_check=True,
            )

    # ---------------- tiles ----------------
    xt = pool.tile([RS, 16 * 128], F16)       # [r, (bc i)*128 + b]
    xs = pool.tile([128, 2 * I * RS], F16)    # [b%128, bc*1024 + i*128 + r]
    ws = pool.tile([RS, COI], F16)            # [r, c*128 + o*8 + i]
    wst = pool.tile([RS, COI], F16)           # [r, c*128 + i*16 + o]
    wp = pool.tile([RS, COI], F16, name=f"{rp}wp", tag="wp")
    prm = pool.tile([1, 2], F32, name=f"{rp}prm", tag="prm")

    if chain:
        # Serialize this rep behind the previous one's final output: a tiny
        # DMA from out_d into xt creates a WAW overlap with the real xt
        # load, so timing reps measure end-to-end latency.
        poison = out_d[0:16, 0:20].bitcast(F16)
        nc.sync.dma_start(xt[0:16, 0 : poison.shape[1]], poison)

    # activation-table prime: first ACT instruction loads the ln/exp table
    nc.vector.memset(prm[:], 1.0)
    nc.scalar.activation(prm[0:1, 0:1], prm[0:1, 0:1],
                         mybir.ActivationFunctionType.Ln)

    nc.sync.dma_start(ws[:], ws_d[:])
    for g in range(4):
        nc.sync.dma_start(
            xt[:, g * 512 : (g + 1) * 512], xt_d[:, g * 512 : (g + 1) * 512]
        )
    nc.sync.dma_start(xs[:], xs_d[:])
    nc.sync.dma_start(wst[:], wst_d[:])

    w4 = ws[:].rearrange("p (c o i) -> p c o i", c=C, o=O, i=I)
    wp4 = wp[:].rearrange("p (c o i) -> p c o i", c=C, o=O, i=I)

    dummy_ps = (
        ps.tile([128, 256], F32, tag="dummy_ps", name=f"{rp}dummy_ps")
        if pace else None
    )

    # collective buffers (HBM).  Payloads are [128, 2*CO] (both batch
    # halves in the free dim) so AG/A2A concatenate along the partition
    # axis in 640-byte rows — keeps DMA descriptors >= 512B (2x faster
    # than 320B rows on the gather side).
    ag_in = [dram.tile([128, 2 * CO], F16, name=f"{rp}ag_in{t}") for t in range(2)]
    ag_out = [
        dram.tile([128 * NCORES, 2 * CO], F16, name=f"{rp}ag_out{t}",
                  addr_space="Shared")
        for t in range(2)
    ]
    a2a_in = dram.tile([128, 2 * CO], F16, name=f"{rp}a2a_in")
    a2a_out = dram.tile([128, 2 * CO], F16, name=f"{rp}a2a_out")

    b_cum = pool.tile([RS, C], F32, name=f"{rp}bcum", tag="bcum")

    if niter == 0:
        nc.sync.dma_start(out_d[:], xt[0:16, 0 : 4 * CO].bitcast(F32))
        return

    for t in range(niter):
        last = t == niter - 1
        # ---- route-weighted sum matmuls ----
        rhs4 = w4 if t == 0 else wp4
        s_ps = [
            ps.tile([128, CO], F32, tag=f"s_ps{bc}", name=f"{rp}s_ps{bc}_{t}")
            for bc in range(2)
        ]
        for bc in range(2):
            for i in range(I):
                nc.tensor.matmul(
                    s_ps[bc][:],
                    xt[:, (bc * 8 + i) * 128 : (bc * 8 + i + 1) * 128],
                    rhs4[:, :, :, i],
                    start=(i == 0),
                    stop=(i == I - 1),
                )
        cat = pool.tile([128, 2 * CO], F16, tag="cat", name=f"{rp}cat_{t}")
        nc.vector.tensor_copy(cat[:, 0:CO], s_ps[0][:])
        nc.scalar.copy(cat[:, CO : 2 * CO], s_ps[1][:])

        if not last:
            # ================= AllGather + tree-reduce =================
            # bc0 half uploads while the bc1 matmuls are still running
            nc.sync.dma_start(ag_in[t][:, 0:CO], cat[:, 0:CO])
            nc.sync.dma_start(ag_in[t][:, CO : 2 * CO], cat[:, CO : 2 * CO])
            if fake_cc:
                # same gather byte-count as the real path, sourced locally
                s_all = pool.tile([128, 16 * CO], F16, tag="s_all",
                                  name=f"{rp}s_all_{t}")
                for rk in range(NCORES):
                    nc.sync.dma_start(
                        s_all[:, rk * 2 * CO : (rk + 1) * 2 * CO],
                        ag_in[t][:],
                    )
            else:
                nc.gpsimd.collective_compute(
                    "AllGather",
                    mybir.AluOpType.bypass,
                    replica_groups=rg,
                    ins=[ag_in[t][:].opt()],
                    outs=[ag_out[t][:].opt()],
                )
                if pace:
                    _pace_pe(dummy_ps, xt, pace_ag)
                s_all = pool.tile([128, 16 * CO], F16, tag="s_all",
                                  name=f"{rp}s_all_{t}")
                # gather rank partials: cols = (rank, bc, co)
                ag_v = ag_out[t][:].rearrange(
                    "(rk p) w -> p rk w", rk=NCORES
                )
                for h in range(2):
                    nc.sync.dma_start(
                        s_all[:, h * 8 * CO : (h + 1) * 8 * CO].rearrange(
                            "p (rk w) -> p rk w", rk=4
                        ),
                        ag_v[:, h * 4 : (h + 1) * 4],
                    )
            # binary tree: 8 -> 4 -> 2 -> 1 rank partials
            s_t1 = pool.tile([128, 8 * CO], F16, tag="s_t1", name=f"{rp}st1_{t}")
            s_t2 = pool.tile([128, 4 * CO], F16, tag="s_t2", name=f"{rp}st2_{t}")
            s_sb = pool.tile([128, 2 * CO], F16, tag="s_sb", name=f"{rp}ssb_{t}")
            nc.vector.tensor_tensor(
                s_t1[:], s_all[:, 0 : 8 * CO], s_all[:, 8 * CO : 16 * CO],
                op=mybir.AluOpType.add,
            )
            nc.vector.tensor_tensor(
                s_t2[:], s_t1[:, 0 : 4 * CO], s_t1[:, 4 * CO : 8 * CO],
                op=mybir.AluOpType.add,
            )
            nc.vector.tensor_tensor(
                s_sb[:], s_t2[:, 0 : 2 * CO], s_t2[:, 2 * CO : 4 * CO],
                op=mybir.AluOpType.add,
            )

            # ---- squash via ln/exp (no sqrt-table switch) ----
            # v = s * a2*sqrt(q)/(1 + a2*q), q = sum_o s^2, a2 = alpha^2
            a2 = A0 * A0 if t == 0 else 1.0
            g = 2 * C
            v_sb = pool.tile([128, 2 * CO], F16, tag="v_sb", name=f"{rp}v_{t}")
            tsq = pool.tile([128, 2 * CO], F32, tag="tsq", name=f"{rp}tsq_{t}")
            q = pool.tile([128, g], F32, tag="sq_q", name=f"{rp}q_{t}")
            u2 = pool.tile([128, g], F32, tag="sq_u", name=f"{rp}u_{t}")
            den = pool.tile([128, g], F32, tag="sq_d", name=f"{rp}d_{t}")
            rw = pool.tile([128, g], F32, tag="sq_r", name=f"{rp}r_{t}")
            gf = pool.tile([128, g], F32, tag="sq_g", name=f"{rp}g_{t}")
            nc.vector.tensor_mul(tsq[:], s_sb[:], s_sb[:])
            nc.vector.tensor_reduce(
                q[:], tsq[:].rearrange("p (g o) -> p g o", o=O),
                axis=mybir.AxisListType.X, op=mybir.AluOpType.add,
            )
            # ln then exp(0.5*h + ln a2) = a2*sqrt(q); same ACT table as
            # the softmax exp below.
            nc.scalar.activation(u2[:], q[:], mybir.ActivationFunctionType.Ln)
            if a2 == 1.0:
                bias_a2 = 0.0
            else:
                lnb = pool.tile([128, 1], F32, name=f"{rp}lnb_{t}", tag="lnb")
                nc.vector.memset(lnb[:], float(math.log(a2)))
                bias_a2 = lnb[:]
            nc.scalar.activation(
                u2[:], u2[:], mybir.ActivationFunctionType.Exp,
                bias=bias_a2, scale=0.5,
            )
            nc.vector.tensor_scalar(
                den[:], q[:], a2, 1.0, mybir.AluOpType.mult, mybir.AluOpType.add
            )
            nc.vector.reciprocal(rw[:], den[:])
            nc.vector.tensor_mul(gf[:], u2[:], rw[:])
            # per-bc v so the bc0 G matmuls can start before bc1 finishes
            for bc in range(2):
                nc.vector.tensor_mul(
                    v_sb[:, bc * CO : (bc + 1) * CO].rearrange(
                        "p (g o) -> p g o", o=O
                    ),
                    s_sb[:, bc * CO : (bc + 1) * CO].rearrange(
                        "p (g o) -> p g o", o=O
                    ),
                    gf[:, bc * C : (bc + 1) * C]
                    .unsqueeze(2)
                    .broadcast_to((128, C, O)),
                )

            # ---- G[r, i, c, o] = sum_b x[b,r,i] * v[b,c,o] ----
            g_ps = [
                pg.tile([128, 3 * CO], F32, tag=f"g_ps{gg}",
                        name=f"{rp}g_ps{gg}_{t}")
                for gg in range(3)
            ]
            for i in range(I):
                out_ap = g_ps[i // 3][:, (i % 3) * CO : (i % 3 + 1) * CO]
                for bc in range(2):
                    nc.tensor.matmul(
                        out_ap,
                        xs[:, bc * 1024 + i * 128 : bc * 1024 + (i + 1) * 128],
                        v_sb[:, bc * CO : (bc + 1) * CO],
                        start=(bc == 0),
                        stop=(bc == 1),
                    )
            if pace:
                _pace_pe(dummy_ps, xt, pace_mid)

            # ---- agree[r,c] = sum_{o,i} W[r,c,o,i] * G[r,i,c,o] ----
            # Stage G out of PSUM into fp16 SBUF on the (otherwise idle)
            # scalar engine as each i-group of matmuls completes; the
            # multiply against wst is then a single packed-fp16 2x DVE op.
            Gs = pool.tile([128, COI], F16, name=f"{rp}Gs_{t}", tag="Gs")
            Gv = Gs[:].rearrange("p (c i o) -> p i c o", c=C, i=I, o=O)
            for gg in range(3):
                ni = 3 if gg < 2 else 2
                nc.scalar.copy(
                    Gv[:, gg * 3 : gg * 3 + ni],
                    g_ps[gg][:, 0 : ni * CO].rearrange(
                        "p (i c o) -> p i c o", i=ni, c=C, o=O
                    ),
                )
            tmpA = pool.tile([128, COI], F16, name=f"{rp}tmpA_{t}", tag="tmpA")
            nc.vector.tensor_mul(tmpA[:], wst[:], Gs[:])
            # fold i-halves (2x packed), then group-reduce per capsule
            tf = pool.tile([128, COI // 2], F16, name=f"{rp}tf_{t}", tag="tf")
            tA4 = tmpA[:].rearrange("p (c i o) -> p c i o", c=C, i=I, o=O)
            nc.vector.tensor_tensor(
                tf[:].rearrange("p (c i o) -> p c i o", c=C, i=I // 2, o=O),
                tA4[:, :, 0 : I // 2], tA4[:, :, I // 2 : I],
                op=mybir.AluOpType.add,
            )
            agree = pool.tile([128, C], F32, name=f"{rp}agree_{t}", tag="agree")
            nc.vector.tensor_reduce(
                agree[:], tf[:].rearrange("p (c io) -> p c io", c=C),
                axis=mybir.AxisListType.X, op=mybir.AluOpType.add,
            )
            # ---- b update (raw sums; 1/B folded into the exp scale) ----
            if t == 0:
                nc.vector.tensor_copy(b_cum[:], agree[:])
            else:
                nc.vector.tensor_tensor(
                    b_cum[:], b_cum[:], agree[:], op=mybir.AluOpType.add
                )
            # ---- c = softmax(b/B) over C; wp = c * W ----
            e_sb = pool.tile([RS, C], F32, name=f"{rp}e_{t}", tag="e_sb")
            se = pool.tile([RS, 1], F32, name=f"{rp}se_{t}", tag="se")
            rse = pool.tile([RS, 1], F32, name=f"{rp}rse_{t}", tag="rse")
            c8 = pool.tile([RS, C * I], F16, name=f"{rp}c8_{t}", tag="c8")
            nc.scalar.activation(
                e_sb[:], b_cum[:], mybir.ActivationFunctionType.Exp,
                bias=0.0, scale=1.0 / B, accum_out=se[:],
            )
            nc.vector.reciprocal(rse[:], se[:])
            # c8[p, c, i-rep] = e*rse broadcast-expanded in one op
            nc.vector.tensor_scalar_mul(
                c8[:].rearrange("p (c i) -> p c i", c=C),
                e_sb[:].unsqueeze(2).broadcast_to((RS, C, I)),
                rse[:],
            )
            # wp[p, c, o, i] = W * c (c8 gives packed innermost i => 2x DVE)
            nc.vector.tensor_mul(
                wp4,
                w4,
                c8[:].rearrange("p (c i) -> p c i", c=C)
                .unsqueeze(2)
                .broadcast_to((RS, C, O, I)),
            )

        else:
            # ================= final: AllToAll + local reduce ==========
            # A2A shards the [128, 320] payload into 8x16 partition rows;
            # this core ends up with rows 16k..16k+15 of every rank = its
            # 32 batches as [16, (bc, c, o)].  kernel() un-permutes on the
            # host.
            nc.sync.dma_start(a2a_in[:, 0:CO], cat[:, 0:CO])
            nc.sync.dma_start(a2a_in[:, CO : 2 * CO], cat[:, CO : 2 * CO])
            if fake_cc:
                nc.sync.dma_start(a2a_out[:], a2a_in[:])
            else:
                nc.gpsimd.collective_compute(
                    "AllToAll",
                    mybir.AluOpType.bypass,
                    replica_groups=rg,
                    ins=[a2a_in[:].opt()],
                    outs=[a2a_out[:].opt()],
                )
            if pace:
                _pace_pe(dummy_ps, xt, pace_a2a)
            FB = 16                      # partition rows per rank shard
            W2 = 2 * CO                  # 320
            sf = pool.tile([FB, 8 * W2], F16, tag="sf")
            nc.sync.dma_start(
                sf[:].rearrange("p (rk w) -> p rk w", rk=8),
                a2a_out[:].rearrange("(rk p) w -> p rk w", rk=8),
            )
            f1 = pool.tile([FB, 4 * W2], F16, tag="f1")
            f2 = pool.tile([FB, 2 * W2], F16, tag="f2")
            s_f = pool.tile([FB, W2], F16, tag="s_f")
            nc.vector.tensor_tensor(
                f1[:], sf[:, 0 : 4 * W2], sf[:, 4 * W2 : 8 * W2],
                op=mybir.AluOpType.add,
            )
            nc.vector.tensor_tensor(
                f2[:], f1[:, 0 : 2 * W2], f1[:, 2 * W2 : 4 * W2],
                op=mybir.AluOpType.add,
            )
            nc.vector.tensor_tensor(
                s_f[:], f2[:, 0:W2], f2[:, W2 : 2 * W2],
                op=mybir.AluOpType.add,
            )
            # exact squash; sqrt(q) = exp(0.5*ln q) keeps the single table
            g2 = 2 * C
            tq = pool.tile([FB, W2], F32, tag="ftq")
            qf = pool.tile([FB, g2], F32, tag="fq")
            uf = pool.tile([FB, g2], F32, tag="fu")
            dn = pool.tile([FB, g2], F32, tag="fd")
            rwf = pool.tile([FB, g2], F32, tag="fr")
            gff = pool.tile([FB, g2], F32, tag="fg")
            vf = pool.tile([FB, W2], F32, tag="fv")
            nc.vector.tensor_mul(tq[:], s_f[:], s_f[:])
            nc.vector.tensor_reduce(
                qf[:], tq[:].rearrange("p (g o) -> p g o", o=O),
                axis=mybir.AxisListType.X, op=mybir.AluOpType.add,
            )
            nc.scalar.activation(uf[:], qf[:], mybir.ActivationFunctionType.Ln)
            nc.scalar.activation(uf[:], uf[:],
                                 mybir.ActivationFunctionType.Exp, scale=0.5)
            nc.vector.tensor_scalar(
                dn[:], qf[:], 1.0, 1.0, mybir.AluOpType.mult,
                mybir.AluOpType.add,
            )
            nc.vector.reciprocal(rwf[:], dn[:])
            nc.vector.tensor_mul(gff[:], uf[:], rwf[:])
            nc.vector.tensor_mul(
                vf[:].rearrange("p (g o) -> p g o", o=O),
                s_f[:].rearrange("p (g o) -> p g o", o=O),
                gff[:].unsqueeze(2).broadcast_to((FB, g2, O)),
            )
            nc.sync.dma_start(out_d[:], vf[:])


_NC_CACHE = {}


def _get_nc():
    if "nc" not in _NC_CACHE:
        _NC_CACHE["nc"] = build_nc()
    return _NC_CACHE["nc"]


def _get_runner():
    """Compile once; reuse the jitted SPMD callable across kernel() calls."""
    if "runner" in _NC_CACHE:
        return _NC_CACHE["runner"]
    import jax
    from jax.sharding import Mesh, PartitionSpec
    from jax.experimental.shard_map import shard_map
    from concourse import bass2jax

    nc = _get_nc()
    bass2jax.install_neuronx_cc_hook()
    partition_name = (
        nc.partition_id_tensor.name if nc.partition_id_tensor else None
    )
    in_names, out_names, out_avals, zero_outs = [], [], [], []
    for alloc in nc.m.functions[0].allocations:
        if not isinstance(alloc, mybir.MemoryLocationSet):
            continue
        name = alloc.memorylocations[0].name
        if alloc.kind == "ExternalInput":
            if name != partition_name:
                in_names.append(name)
        elif alloc.kind == "ExternalOutput":
            out_names.append(name)
            shape = tuple(alloc.tensor_shape)
            dtype = mybir.dt.np(alloc.dtype)
            out_avals.append(jax.core.ShapedArray(shape, dtype))
            zero_outs.append(np.zeros(shape, dtype))
    n_params = len(in_names)
    n_outs = len(out_avals)
    all_in_names = list(in_names) + list(out_names)
    if partition_name is not None:
        all_in_names.append(partition_name)

    def _body(*args):
        operands = list(args)
        if partition_name is not None:
            operands.append(bass2jax.partition_id_tensor())
        outs = bass2jax._bass_exec_p.bind(
            *operands,
            out_avals=tuple(out_avals),
            in_names=tuple(all_in_names),
            out_names=tuple(out_names),
            lowering_input_output_aliases=(),
            sim_require_finite=True,
            sim_require_nnan=True,
            nc=nc,
        )
        return tuple(outs)

    devices = jax.devices()[:NCORES]
    mesh = Mesh(np.asarray(devices), ("core",))
    in_specs = (PartitionSpec("core"),) * (n_params + n_outs)
    out_specs = (PartitionSpec("core"),) * len(out_names)
    donate = tuple(range(n_params, n_params + n_outs))
    sharded = jax.jit(
        shard_map(_body, mesh=mesh, in_specs=in_specs, out_specs=out_specs,
                  check_rep=False),
        donate_argnums=donate,
        keep_unused=True,
    )

    def run(in_maps):
        concat_in = [
            np.concatenate(
                [np.asarray(in_maps[c][in_names[i]]) for c in range(NCORES)],
                axis=0,
            )
            for i in range(n_params)
        ]
        concat_zeros = [
            np.zeros((NCORES * z.shape[0], *z.shape[1:]), z.dtype)
            for z in zero_outs
        ]
        out_arrs = sharded(*concat_in, *concat_zeros)
        return [
            {
                name: np.asarray(out_arrs[i]).reshape(
                    NCORES, *out_avals[i].shape
                )[c]
                for i, name in enumerate(out_names)
            }
            for c in range(NCORES)
        ]

    _NC_CACHE["runner"] = run
    return run


def make_in_maps(x, W):
    x = np.asarray(x, dtype=np.float32).astype(np.float16)
    W = np.asarray(W, dtype=np.float32).astype(np.float16)
    in_maps = []
    for k in range(NCORES):
        sl = slice(k * RS, (k + 1) * RS)
        x4 = x[:, :, sl].reshape(2, 128, I, RS)          # [bc, b, i, r]
        xs = np.ascontiguousarray(x4.transpose(1, 0, 2, 3)).reshape(
            128, 2 * I * RS
        )                                                # [b, (bc i r)]
        xt = np.ascontiguousarray(x4.transpose(3, 0, 2, 1)).reshape(
            RS, 2 * I * 128
        )                                                # [r, (bc i b)]
        ws = np.ascontiguousarray(W[sl]).reshape(RS, COI)
        wst = np.ascontiguousarray(W[sl].transpose(0, 1, 3, 2)).reshape(
            RS, COI
        )                                                # [r, (c i o)]
        in_maps.append({"xt": xt, "xs": xs, "ws": ws, "wst": wst})
    return in_maps


def kernel(x, W):
    in_maps = make_in_maps(x, W)
    results = None
    for attempt in range(2):
        try:
            run = _get_runner()
            results = run(in_maps)
            break
        except Exception:
            # Transient device wedges (NRT_EXEC_UNIT_UNRECOVERABLE) have
            # been observed to recover on a fresh attempt; rebuild the
            # compiled runner once before giving up.
            if attempt == 1:
                raise
            _NC_CACHE.clear()
    # un-permute the A2A shard layout: core k row r half h = batch
    # h*128 + 16k + r
    v = np.stack([r["vout"] for r in results]).reshape(NCORES, 16, 2, CO)
    full = np.empty((B, CO), np.float32)
    for h in range(2):
        full[h * 128 : (h + 1) * 128] = v[:, :, h].reshape(128, CO)
    return full.reshape(B, C, O, 1)


if __name__ == "__main__":
    nc = build_nc()
    print("built ok; instructions:",
          sum(len(bb.instructions) for bb in nc.main_func.blocks))


# revision 37
# speedup vs baseline: 2.0543x; 2.0543x over previous
"""DigitCaps dynamic-routing kernel for 8 Trainium2 NeuronCores.

Strategy (v2): shard the routes dimension R=1024 across the 8 cores (128
routes per core).  u_hat is never materialized: each routing iteration
computes its weighted route-sum

    s[b,c,o] = sum_{r,i} x[b,r,i] * (c_ij[r,c] * W[r,c,o,i])

directly on the PE as 16 accumulating fp16 matmuls.  The per-core partial
s is combined across cores with an AllGather + on-core binary-tree adds
(iters 0,1; ~2x cheaper than AllReduce on one chip) and an AllToAll +
local tree-reduce for the final iteration (each core then squashes only
its own 32-batch output shard).  The agreement update

    agree[r,c] = sum_{b,o,i} W[r,c,o,i] * x[b,r,i] * v[b,c,o]

is computed locally via G[r,i,c,o] = sum_b x[b,r,i]*v[b,c,o] (PE matmuls)
followed by a multiply + group-reduce on the vector engine.

Activation-table discipline: iters 0/1 compute sqrt(q) as exp(0.5*ln q)
so the scalar engine only ever needs the ln/exp table (shared with the
softmax exp); the exact-sqrt table for the final squash is pre-loaded by
a dummy op that hides under the AllToAll window.  x is supplied twice
from the host (b-major and r-major) so no on-device transposes are
needed.
"""

import math
import sys

for _p in ("/opt/trn_rl_repo",):
    if _p not in sys.path:
        sys.path.insert(0, _p)

import numpy as np

import concourse.bass as bass
import concourse.bacc as bacc
import concourse.mybir as mybir
import concourse.tile as tile
from concourse.bass_utils import run_bass_kernel_spmd

F32 = mybir.dt.float32
F16 = mybir.dt.float16

B, R, C, O, I = 256, 1024, 10, 16, 8
NCORES = 8
RS = R // NCORES          # routes per core
CO = C * O                # 160
COI = C * O * I           # 1280
BS = B // NCORES          # output batch shard per core
NITER = 3
A0 = 0.1                  # softmax(0) over C=10 entries
# pacer counts target ~65% of the modeled PE-idle window: undershoot only
# costs the 1.2GHz->2.4GHz delta on the next matmul block, while overshoot
# delays it outright (the dummies queue in-order ahead of it)
PACE_AG = 62              # mm-end .. G-start window (~10us modeled)
PACE_A2A = 20             # AllToAll window (nothing queued behind)
PACE_MID = 30             # agree/softmax stretch (~5us modeled)


def build_nc(reps=1, niter=NITER, fake_cc=False, chain=False, pace=True,
             pace_ag=PACE_AG, pace_mid=PACE_MID, pace_a2a=PACE_A2A):
    nc = bacc.Bacc(
        "TRN2", target_bir_lowering=False, debug=False, num_devices=NCORES
    )
    xt_d = nc.dram_tensor("xt", [RS, 2 * I * 128], F16, kind="ExternalInput")
    xs_d = nc.dram_tensor("xs", [128, 2 * I * RS], F16, kind="ExternalInput")
    ws_d = nc.dram_tensor("ws", [RS, COI], F16, kind="ExternalInput")
    # W in (c, i, o) layout: packed innermost match for the agree multiply
    wst_d = nc.dram_tensor("wst", [RS, COI], F16, kind="ExternalInput")
    # [16, 2*CO]: row r = batches (16k + r, 128 + 16k + r) for core k —
    # the A2A partition-shard layout; kernel() un-permutes on the host.
    out_d = nc.dram_tensor("vout", [16, 2 * CO], F32, kind="ExternalOutput")

    with tile.TileContext(nc) as tc:
        with (
            tc.tile_pool(name="main", bufs=1) as pool,
            tc.tile_pool(name="ps", bufs=1, space=bass.MemorySpace.PSUM) as ps,
            tc.tile_pool(name="pg", bufs=1, space=bass.MemorySpace.PSUM) as pg,
            tc.tile_pool(name="dram", bufs=1, space="DRAM") as dram,
        ):
            pools = (pool, ps, pg, dram)
            for rep in range(reps):
                _build_body(nc, tc, pools, xt_d, xs_d, ws_d, wst_d, out_d, rep,
                            niter=niter, fake_cc=fake_cc,
                            chain=chain and rep > 0, pace=pace,
                            pace_ag=pace_ag, pace_mid=pace_mid,
                            pace_a2a=pace_a2a)
    nc.finalize()
    _unify_act_tables(nc)
    return nc


def _unify_act_tables(nc):
    """Point every activation-table load at the one table covering all the
    functions this kernel uses (ln, exp, copy), then drop redundant loads.

    The builtin insertion pass picks the first table containing each
    function (ln -> natural_log, exp -> exp_and_others), which forces a
    ~1.3us table reload at every ln<->exp transition on the scalar engine.
    All loads it inserts are dependency-free queue-order instructions, so
    rewriting ids and deleting duplicates is safe.
    """
    from concourse.hw_specs import get_activation_tables

    need = {
        mybir.ActivationFunctionType.Ln,
        mybir.ActivationFunctionType.Exp,
        mybir.ActivationFunctionType.Copy,
    }
    try:
        tabs = get_activation_tables(nc.m.arch)
        names = list(tabs)
        target = names.index("natural_log_exp_and_others")
        if not need <= tabs["natural_log_exp_and_others"]:
            return
    except Exception:
        # unknown act_info layout: leave the stock (slower) table loads
        return
    cur = None
    for bb in nc.main_func.blocks:
        keep = []
        for inst in bb.instructions:
            if isinstance(inst, mybir.InstLoadActFuncSet):
                inst.act_func_set_id = target
                if cur == target:
                    continue
                cur = target
            elif isinstance(inst, mybir.InstActivation):
                assert inst.func in need, f"unexpected ACT func {inst.func}"
            keep.append(inst)
        bb.instructions[:] = keep


def _build_body(nc, tc, pools, xt_d, xs_d, ws_d, wst_d, out_d, rep, niter=NITER,
                fake_cc=False, chain=False, pace=True, pace_ag=PACE_AG,
                pace_mid=PACE_MID, pace_a2a=PACE_A2A):
    pool, ps, pg, dram = pools
    rg = [list(range(NCORES))]
    rp = f"r{rep}_"

    def _pace_pe(dummy_ps, xt, n):
        # Keep the PE p-state streak alive across collective/DVE windows:
        # self-paced throwaway matmuls into a scratch PSUM bank.  No
        # consumers; they only read xt, so they fill PE idle time without
        # delaying ready work by more than one dummy's tail (~110ns).
        for _ in range(n):
            nc.tensor.matmul(
                dummy_ps[:], xt[:, 0:128], xt[:, 0:256],
                start=True, stop=True, skip_group_check=True,
            )

    # ---------------- tiles ----------------
    xt = pool.tile([RS, 16 * 128], F16)       # [r, (bc i)*128 + b]
    xs = pool.tile([128, 2 * I * RS], F16)    # [b%128, bc*1024 + i*128 + r]
    ws = pool.tile([RS, COI], F16)            # [r, c*128 + o*8 + i]
    wst = pool.tile([RS, COI], F16)           # [r, c*128 + i*16 + o]
    wp = pool.tile([RS, COI], F16, name=f"{rp}wp", tag="wp")
    prm = pool.tile([1, 2], F32, name=f"{rp}prm", tag="prm")

    if chain:
        # Serialize this rep behind the previous one's final output: a tiny
        # DMA from out_d into xt creates a WAW overlap with the real xt
        # load, so timing reps measure end-to-end latency.
        poison = out_d[0:16, 0:20].bitcast(F16)
        nc.sync.dma_start(xt[0:16, 0 : poison.shape[1]], poison)

    # activation-table prime: first ACT instruction loads the ln/exp table
    nc.vector.memset(prm[:], 1.0)
    nc.scalar.activation(prm[0:1, 0:1], prm[0:1, 0:1],
                         mybir.ActivationFunctionType.Ln)

    nc.sync.dma_start(ws[:], ws_d[:])
    for g in range(4):
        nc.sync.dma_start(
            xt[:, g * 512 : (g + 1) * 512], xt_d[:, g * 512 : (g + 1) * 512]
        )
    nc.sync.dma_start(xs[:], xs_d[:])
    nc.sync.dma_start(wst[:], wst_d[:])

    w4 = ws[:].rearrange("p (c o i) -> p c o i", c=C, o=O, i=I)
    wp4 = wp[:].rearrange("p (c o i) -> p c o i", c=C, o=O, i=I)

    dummy_ps = (
        ps.tile([128, 256], F32, tag="dummy_ps", name=f"{rp}dummy_ps")
        if pace else None
    )

    # collective buffers (HBM).  Payloads are [128, 2*CO] (both batch
    # halves in the free dim) so AG/A2A concatenate along the partition
    # axis in 640-byte rows — keeps DMA descriptors >= 512B (2x faster
    # than 320B rows on the gather side).
    ag_in = [dram.tile([128, 2 * CO], F16, name=f"{rp}ag_in{t}") for t in range(2)]
    ag_out = [
        dram.tile([128 * NCORES, 2 * CO], F16, name=f"{rp}ag_out{t}",
                  addr_space="Shared")
        for t in range(2)
    ]
    a2a_in = dram.tile([128, 2 * CO], F16, name=f"{rp}a2a_in")
    a2a_out = dram.tile([128, 2 * CO], F16, name=f"{rp}a2a_out")

    b_cum = pool.tile([RS, C], F32, name=f"{rp}bcum", tag="bcum")

    if niter == 0:
        nc.sync.dma_start(out_d[:], xt[0:16, 0 : 4 * CO].bitcast(F32))
        return

    for t in range(niter):
        last = t == niter - 1
        # ---- route-weighted sum matmuls ----
        rhs4 = w4 if t == 0 else wp4
        s_ps = [
            ps.tile([128, CO], F32, tag=f"s_ps{bc}", name=f"{rp}s_ps{bc}_{t}")
            for bc in range(2)
        ]
        for bc in range(2):
            for i in range(I):
                nc.tensor.matmul(
                    s_ps[bc][:],
                    xt[:, (bc * 8 + i) * 128 : (bc * 8 + i + 1) * 128],
                    rhs4[:, :, :, i],
                    start=(i == 0),
                    stop=(i == I - 1),
                )
        cat = pool.tile([128, 2 * CO], F16, tag="cat", name=f"{rp}cat_{t}")
        nc.vector.tensor_copy(cat[:, 0:CO], s_ps[0][:])
        nc.scalar.copy(cat[:, CO : 2 * CO], s_ps[1][:])

        if not last:
            # ================= AllGather + tree-reduce =================
            # bc0 half uploads while the bc1 matmuls are still running
            nc.sync.dma_start(ag_in[t][:, 0:CO], cat[:, 0:CO])
            nc.sync.dma_start(ag_in[t][:, CO : 2 * CO], cat[:, CO : 2 * CO])
            if fake_cc:
                # same gather byte-count as the real path, sourced locally
                s_all = pool.tile([128, 16 * CO], F16, tag="s_all",
                                  name=f"{rp}s_all_{t}")
                for rk in range(NCORES):
                    nc.sync.dma_start(
                        s_all[:, rk * 2 * CO : (rk + 1) * 2 * CO],
                        ag_in[t][:],
                    )
            else:
                nc.gpsimd.collective_compute(
                    "AllGather",
                    mybir.AluOpType.bypass,
                    replica_groups=rg,
                    ins=[ag_in[t][:].opt()],
                    outs=[ag_out[t][:].opt()],
                )
                if pace:
                    _pace_pe(dummy_ps, xt, pace_ag)
                s_all = pool.tile([128, 16 * CO], F16, tag="s_all",
                                  name=f"{rp}s_all_{t}")
                # gather rank partials: cols = (rank, bc, co); one DMA —
                # the tree's first add needs all ranks anyway, and one
                # transfer means one semaphore wait
                nc.sync.dma_start(
                    s_all[:].rearrange("p (rk w) -> p rk w", rk=NCORES),
                    ag_out[t][:].rearrange("(rk p) w -> p rk w", rk=NCORES),
                )
            # binary tree: 8 -> 4 -> 2 -> 1 rank partials
            s_t1 = pool.tile([128, 8 * CO], F16, tag="s_t1", name=f"{rp}st1_{t}")
            s_t2 = pool.tile([128, 4 * CO], F16, tag="s_t2", name=f"{rp}st2_{t}")
            s_sb = pool.tile([128, 2 * CO], F16, tag="s_sb", name=f"{rp}ssb_{t}")
            nc.vector.tensor_tensor(
                s_t1[:], s_all[:, 0 : 8 * CO], s_all[:, 8 * CO : 16 * CO],
                op=mybir.AluOpType.add,
            )
            nc.vector.tensor_tensor(
                s_t2[:], s_t1[:, 0 : 4 * CO], s_t1[:, 4 * CO : 8 * CO],
                op=mybir.AluOpType.add,
            )
            nc.vector.tensor_tensor(
                s_sb[:], s_t2[:, 0 : 2 * CO], s_t2[:, 2 * CO : 4 * CO],
                op=mybir.AluOpType.add,
            )

            # ---- squash via ln/exp (no sqrt-table switch) ----
            # v = s * a2*sqrt(q)/(1 + a2*q), q = sum_o s^2, a2 = alpha^2
            a2 = A0 * A0 if t == 0 else 1.0
            g = 2 * C
            v_sb = pool.tile([128, 2 * CO], F16, tag="v_sb", name=f"{rp}v_{t}")
            tsq = pool.tile([128, 2 * CO], F32, tag="tsq", name=f"{rp}tsq_{t}")
            q = pool.tile([128, g], F32, tag="sq_q", name=f"{rp}q_{t}")
            u2 = pool.tile([128, g], F32, tag="sq_u", name=f"{rp}u_{t}")
            den = pool.tile([128, g], F32, tag="sq_d", name=f"{rp}d_{t}")
            rw = pool.tile([128, g], F32, tag="sq_r", name=f"{rp}r_{t}")
            gf = pool.tile([128, g], F32, tag="sq_g", name=f"{rp}g_{t}")
            nc.vector.tensor_mul(tsq[:], s_sb[:], s_sb[:])
            nc.vector.tensor_reduce(
                q[:], tsq[:].rearrange("p (g o) -> p g o", o=O),
                axis=mybir.AxisListType.X, op=mybir.AluOpType.add,
            )
            # ln then exp(0.5*h + ln a2) = a2*sqrt(q); same ACT table as
            # the softmax exp below.
            nc.scalar.activation(u2[:], q[:], mybir.ActivationFunctionType.Ln)
            if a2 == 1.0:
                bias_a2 = 0.0
            else:
                lnb = pool.tile([128, 1], F32, name=f"{rp}lnb_{t}", tag="lnb")
                nc.vector.memset(lnb[:], float(math.log(a2)))
                bias_a2 = lnb[:]
            nc.scalar.activation(
                u2[:], u2[:], mybir.ActivationFunctionType.Exp,
                bias=bias_a2, scale=0.5,
            )
            nc.vector.tensor_scalar(
                den[:], q[:], a2, 1.0, mybir.AluOpType.mult, mybir.AluOpType.add
            )
            nc.vector.reciprocal(rw[:], den[:])
            nc.vector.tensor_mul(gf[:], u2[:], rw[:])
            # per-bc v so the bc0 G matmuls can start before bc1 finishes
            for bc in range(2):
                nc.vector.tensor_mul(
                    v_sb[:, bc * CO : (bc + 1) * CO].rearrange(
                        "p (g o) -> p g o", o=O
                    ),
                    s_sb[:, bc * CO : (bc + 1) * CO].rearrange(
                        "p (g o) -> p g o", o=O
                    ),
                    gf[:, bc * C : (bc + 1) * C]
                    .unsqueeze(2)
                    .broadcast_to((128, C, O)),
                )

            # ---- G[r, i, c, o] = sum_b x[b,r,i] * v[b,c,o] ----
            g_ps = [
                pg.tile([128, 3 * CO], F32, tag=f"g_ps{gg}",
                        name=f"{rp}g_ps{gg}_{t}")
                for gg in range(3)
            ]
            for i in range(I):
                out_ap = g_ps[i // 3][:, (i % 3) * CO : (i % 3 + 1) * CO]
                for bc in range(2):
                    nc.tensor.matmul(
                        out_ap,
                        xs[:, bc * 1024 + i * 128 : bc * 1024 + (i + 1) * 128],
                        v_sb[:, bc * CO : (bc + 1) * CO],
                        start=(bc == 0),
                        stop=(bc == 1),
                    )
            if pace:
                _pace_pe(dummy_ps, xt, pace_mid)

            # ---- agree[r,c] = sum_{o,i} W[r,c,o,i] * G[r,i,c,o] ----
            # Stage G out of PSUM into fp16 SBUF on the (otherwise idle)
            # scalar engine as each i-group of matmuls completes; the
            # multiply against wst is then a single packed-fp16 2x DVE op.
            Gs = pool.tile([128, COI], F16, name=f"{rp}Gs_{t}", tag="Gs")
            Gv = Gs[:].rearrange("p (c i o) -> p i c o", c=C, i=I, o=O)
            for gg in range(3):
                ni = 3 if gg < 2 else 2
                nc.scalar.copy(
                    Gv[:, gg * 3 : gg * 3 + ni],
                    g_ps[gg][:, 0 : ni * CO].rearrange(
                        "p (i c o) -> p i c o", i=ni, c=C, o=O
                    ),
                )
            tmpA = pool.tile([128, COI], F16, name=f"{rp}tmpA_{t}", tag="tmpA")
            nc.vector.tensor_mul(tmpA[:], wst[:], Gs[:])
            # fold i-halves (2x packed), then group-reduce per capsule
            tf = pool.tile([128, COI // 2], F16, name=f"{rp}tf_{t}", tag="tf")
            tA4 = tmpA[:].rearrange("p (c i o) -> p c i o", c=C, i=I, o=O)
            nc.vector.tensor_tensor(
                tf[:].rearrange("p (c i o) -> p c i o", c=C, i=I // 2, o=O),
                tA4[:, :, 0 : I // 2], tA4[:, :, I // 2 : I],
                op=mybir.AluOpType.add,
            )
            agree = pool.tile([128, C], F32, name=f"{rp}agree_{t}", tag="agree")
            nc.vector.tensor_reduce(
                agree[:], tf[:].rearrange("p (c io) -> p c io", c=C),
                axis=mybir.AxisListType.X, op=mybir.AluOpType.add,
            )
            # ---- b update (raw sums; 1/B folded into the exp scale) ----
            if t == 0:
                nc.vector.tensor_copy(b_cum[:], agree[:])
            else:
                nc.vector.tensor_tensor(
                    b_cum[:], b_cum[:], agree[:], op=mybir.AluOpType.add
                )
            # ---- c = softmax(b/B) over C; wp = c * W ----
            e_sb = pool.tile([RS, C], F32, name=f"{rp}e_{t}", tag="e_sb")
            se = pool.tile([RS, 1], F32, name=f"{rp}se_{t}", tag="se")
            rse = pool.tile([RS, 1], F32, name=f"{rp}rse_{t}", tag="rse")
            c8 = pool.tile([RS, C * I], F16, name=f"{rp}c8_{t}", tag="c8")
            nc.scalar.activation(
                e_sb[:], b_cum[:], mybir.ActivationFunctionType.Exp,
                bias=0.0, scale=1.0 / B, accum_out=se[:],
            )
            nc.vector.reciprocal(rse[:], se[:])
            # c8[p, c, i-rep] = e*rse broadcast-expanded in one op
            nc.vector.tensor_scalar_mul(
                c8[:].rearrange("p (c i) -> p c i", c=C),
                e_sb[:].unsqueeze(2).broadcast_to((RS, C, I)),
                rse[:],
            )
            # wp[p, c, o, i] = W * c (c8 gives packed innermost i => 2x DVE)
            nc.vector.tensor_mul(
                wp4,
                w4,
                c8[:].rearrange("p (c i) -> p c i", c=C)
                .unsqueeze(2)
                .broadcast_to((RS, C, O, I)),
            )

        else:
            # ================= final: AllToAll + local reduce ==========
            # A2A shards the [128, 320] payload into 8x16 partition rows;
            # this core ends up with rows 16k..16k+15 of every rank = its
            # 32 batches as [16, (bc, c, o)].  kernel() un-permutes on the
            # host.
            nc.sync.dma_start(a2a_in[:, 0:CO], cat[:, 0:CO])
            nc.sync.dma_start(a2a_in[:, CO : 2 * CO], cat[:, CO : 2 * CO])
            if fake_cc:
                nc.sync.dma_start(a2a_out[:], a2a_in[:])
            else:
                nc.gpsimd.collective_compute(
                    "AllToAll",
                    mybir.AluOpType.bypass,
                    replica_groups=rg,
                    ins=[a2a_in[:].opt()],
                    outs=[a2a_out[:].opt()],
                )
            if pace:
                _pace_pe(dummy_ps, xt, pace_a2a)
            FB = 16                      # partition rows per rank shard
            W2 = 2 * CO                  # 320
            sf = pool.tile([FB, 8 * W2], F16, tag="sf")
            for h in range(2):
                nc.sync.dma_start(
                    sf[:, h * 4 * W2 : (h + 1) * 4 * W2].rearrange(
                        "p (rk w) -> p rk w", rk=4
                    ),
                    a2a_out[h * 64 : (h + 1) * 64, :].rearrange(
                        "(rk p) w -> p rk w", rk=4
                    ),
                )
            f1 = pool.tile([FB, 4 * W2], F16, tag="f1")
            f2 = pool.tile([FB, 2 * W2], F16, tag="f2")
            s_f = pool.tile([FB, W2], F16, tag="s_f")
            nc.vector.tensor_tensor(
                f1[:], sf[:, 0 : 4 * W2], sf[:, 4 * W2 : 8 * W2],
                op=mybir.AluOpType.add,
            )
            nc.vector.tensor_tensor(
                f2[:], f1[:, 0 : 2 * W2], f1[:, 2 * W2 : 4 * W2],
                op=mybir.AluOpType.add,
            )
            nc.vector.tensor_tensor(
                s_f[:], f2[:, 0:W2], f2[:, W2 : 2 * W2],
                op=mybir.AluOpType.add,
            )
            # exact squash; sqrt(q) = exp(0.5*ln q) keeps the single table
            g2 = 2 * C
            tq = pool.tile([FB, W2], F32, tag="ftq")
            qf = pool.tile([FB, g2], F32, tag="fq")
            uf = pool.tile([FB, g2], F32, tag="fu")
            dn = pool.tile([FB, g2], F32, tag="fd")
            rwf = pool.tile([FB, g2], F32, tag="fr")
            gff = pool.tile([FB, g2], F32, tag="fg")
            vf = pool.tile([FB, W2], F32, tag="fv")
            nc.vector.tensor_mul(tq[:], s_f[:], s_f[:])
            nc.vector.tensor_reduce(
                qf[:], tq[:].rearrange("p (g o) -> p g o", o=O),
                axis=mybir.AxisListType.X, op=mybir.AluOpType.add,
            )
            nc.scalar.activation(uf[:], qf[:], mybir.ActivationFunctionType.Ln)
            nc.scalar.activation(uf[:], uf[:],
                                 mybir.ActivationFunctionType.Exp, scale=0.5)
            nc.vector.tensor_scalar(
                dn[:], qf[:], 1.0, 1.0, mybir.AluOpType.mult,
                mybir.AluOpType.add,
            )
            nc.vector.reciprocal(rwf[:], dn[:])
            nc.vector.tensor_mul(gff[:], uf[:], rwf[:])
            nc.vector.tensor_mul(
                vf[:].rearrange("p (g o) -> p g o", o=O),
                s_f[:].rearrange("p (g o) -> p g o", o=O),
                gff[:].unsqueeze(2).broadcast_to((FB, g2, O)),
            )
            nc.sync.dma_start(out_d[:], vf[:])


_NC_CACHE = {}


def _get_nc():
    if "nc" not in _NC_CACHE:
        _NC_CACHE["nc"] = build_nc()
    return _NC_CACHE["nc"]


def _get_runner():
    """Compile once; reuse the jitted SPMD callable across kernel() calls."""
    if "runner" in _NC_CACHE:
        return _NC_CACHE["runner"]
    import jax
    from jax.sharding import Mesh, PartitionSpec
    from jax.experimental.shard_map import shard_map
    from concourse import bass2jax

    nc = _get_nc()
    bass2jax.install_neuronx_cc_hook()
    partition_name = (
        nc.partition_id_tensor.name if nc.partition_id_tensor else None
    )
    in_names, out_names, out_avals, zero_outs = [], [], [], []
    for alloc in nc.m.functions[0].allocations:
        if not isinstance(alloc, mybir.MemoryLocationSet):
            continue
        name = alloc.memorylocations[0].name
        if alloc.kind == "ExternalInput":
            if name != partition_name:
                in_names.append(name)
        elif alloc.kind == "ExternalOutput":
            out_names.append(name)
            shape = tuple(alloc.tensor_shape)
            dtype = mybir.dt.np(alloc.dtype)
            out_avals.append(jax.core.ShapedArray(shape, dtype))
            zero_outs.append(np.zeros(shape, dtype))
    n_params = len(in_names)
    n_outs = len(out_avals)
    all_in_names = list(in_names) + list(out_names)
    if partition_name is not None:
        all_in_names.append(partition_name)

    def _body(*args):
        operands = list(args)
        if partition_name is not None:
            operands.append(bass2jax.partition_id_tensor())
        outs = bass2jax._bass_exec_p.bind(
            *operands,
            out_avals=tuple(out_avals),
            in_names=tuple(all_in_names),
            out_names=tuple(out_names),
            lowering_input_output_aliases=(),
            sim_require_finite=True,
            sim_require_nnan=True,
            nc=nc,
        )
        return tuple(outs)

    devices = jax.devices()[:NCORES]
    mesh = Mesh(np.asarray(devices), ("core",))
    in_specs = (PartitionSpec("core"),) * (n_params + n_outs)
    out_specs = (PartitionSpec("core"),) * len(out_names)
    donate = tuple(range(n_params, n_params + n_outs))
    sharded = jax.jit(
        shard_map(_body, mesh=mesh, in_specs=in_specs, out_specs=out_specs,
                  check_rep=False),
        donate_argnums=donate,
        keep_unused=True,
    )

    def run(in_maps):
        concat_in = [
            np.concatenate(
                [np.asarray(in_maps[c][in_names[i]]) for c in range(NCORES)],
                axis=0,
            )
            for i in range(n_params)
        ]
        concat_zeros = [
            np.zeros((NCORES * z.shape[0], *z.shape[1:]), z.dtype)
            for z in zero_outs
        ]
        out_arrs = sharded(*concat_in, *concat_zeros)
        return [
            {
                name: np.asarray(out_arrs[i]).reshape(
                    NCORES, *out_avals[i].shape
                )[c]
                for i, name in enumerate(out_names)
            }
            for c in range(NCORES)
        ]

    _NC_CACHE["runner"] = run
    return run


def make_in_maps(x, W):
    x = np.asarray(x, dtype=np.float32).astype(np.float16)
    W = np.asarray(W, dtype=np.float32).astype(np.float16)
    in_maps = []
    for k in range(NCORES):
        sl = slice(k * RS, (k + 1) * RS)
        x4 = x[:, :, sl].reshape(2, 128, I, RS)          # [bc, b, i, r]
        xs = np.ascontiguousarray(x4.transpose(1, 0, 2, 3)).reshape(
            128, 2 * I * RS
        )                                                # [b, (bc i r)]
        xt = np.ascontiguousarray(x4.transpose(3, 0, 2, 1)).reshape(
            RS, 2 * I * 128
        )                                                # [r, (bc i b)]
        ws = np.ascontiguousarray(W[sl]).reshape(RS, COI)
        wst = np.ascontiguousarray(W[sl].transpose(0, 1, 3, 2)).reshape(
            RS, COI
        )                                                # [r, (c i o)]
        in_maps.append({"xt": xt, "xs": xs, "ws": ws, "wst": wst})
    return in_maps


def kernel(x, W):
    in_maps = make_in_maps(x, W)
    results = None
    for attempt in range(2):
        try:
            run = _get_runner()
            results = run(in_maps)
            break
        except Exception:
            # Transient device wedges (NRT_EXEC_UNIT_UNRECOVERABLE) have
            # been observed to recover on a fresh attempt; rebuild the
            # compiled runner once before giving up.
            if attempt == 1:
                raise
            _NC_CACHE.clear()
    # un-permute the A2A shard layout: core k row r half h = batch
    # h*128 + 16k + r
    v = np.stack([r["vout"] for r in results]).reshape(NCORES, 16, 2, CO)
    full = np.empty((B, CO), np.float32)
    for h in range(2):
        full[h * 128 : (h + 1) * 128] = v[:, :, h].reshape(128, CO)
    return full.reshape(B, C, O, 1)


if __name__ == "__main__":
    nc = build_nc()
    print("built ok; instructions:",
          sum(len(bb.instructions) for bb in nc.main_func.blocks))
